# revision 8
# baseline (speedup 1.0000x reference)
"""DenseMatcher kernel for 8 TRN2 NeuronCores.

sim = (q/|q|)^T (p/|p|)  [9216, 9216], row-argmax, col-argmax, mutual-NN.

Sharding: core k owns sim rows [k*1152, (k+1)*1152) (GEMM1: q_shard^T @ p_full)
and sim cols [k*1152, (k+1)*1152) (GEMM2: p_shard^T @ q_full, whose row-argmax
is the col-argmax of sim). All heavy compute (normalize, both GEMMs, argmax
scans) on device; host only slices inputs, concatenates per-core results and
does the O(M) mutual-NN index arithmetic.
"""

import numpy as np
import sys

sys.path.insert(0, "/opt/trn_rl_repo")

import concourse.bacc as bacc
import concourse.bass as bass
import concourse.mybir as mybir
from concourse.tile import TileContext

F32 = mybir.dt.float32
U32 = mybir.dt.uint32

C = 256  # channels (2 partition tiles of 128)
H = 96
W = 96
NTOT = H * W  # 9216
NCORES = 8
NS = NTOT // NCORES  # 1152 rows/cols per core
G = NS // 128  # 9 groups of 128 rows
CHUNK = 512
NCHUNK = NTOT // CHUNK  # 18
THRESHOLD = 0.9


def build_graph(ntot=NTOT, ns=NS, do_compile=True):
    """Build the per-core SPMD graph. Parameterized so a scaled-down version
    can run under the interpreter for shape checking."""
    g = ns // 128
    nchunk_full = (ntot + CHUNK - 1) // CHUNK

    nc = bacc.Bacc("TRN2", target_bir_lowering=False, debug=False)
    q_full = nc.declare_dram_parameter("q_full", [C, ntot], F32, isOutput=False)
    p_full = nc.declare_dram_parameter("p_full", [C, ntot], F32, isOutput=False)
    q_shard = nc.declare_dram_parameter("q_shard", [C, ns], F32, isOutput=False)
    p_shard = nc.declare_dram_parameter("p_shard", [C, ns], F32, isOutput=False)

    sim_out = nc.declare_dram_parameter("sim", [ns, ntot], F32, isOutput=True)
    row_vals_o = nc.declare_dram_parameter("row_vals", [128, g * 8], F32, isOutput=True)
    row_idx_o = nc.declare_dram_parameter("row_idx", [128, g * 8], U32, isOutput=True)
    col_vals_o = nc.declare_dram_parameter("col_vals", [128, g * 8], F32, isOutput=True)
    col_idx_o = nc.declare_dram_parameter("col_idx", [128, g * 8], U32, isOutput=True)

    with TileContext(nc) as tc:
        with (
            tc.tile_pool(name="stream", bufs=1) as pool_stream,
            tc.tile_pool(name="block", bufs=2) as pool_block,
            tc.tile_pool(name="shard", bufs=1) as pool_shard,
            tc.tile_pool(name="small", bufs=2) as pool_small,
            tc.tile_pool(name="sq", bufs=2) as pool_sq,
            tc.tile_pool(name="res", bufs=1) as pool_res,
            tc.tile_pool(name="psum", bufs=8, space="PSUM") as pool_psum,
            tc.tile_pool(name="const", bufs=1) as pool_const,
        ):
            ones = pool_const.tile([128, 128], F32, tag="ones")
            nc.vector.memset(ones, 1.0)

            # result tiles (persist across group loops)
            row_vals = pool_res.tile([128, g * 8], F32, tag="rv")
            row_idx = pool_res.tile([128, g * 8], U32, tag="ri")
            col_vals = pool_res.tile([128, g * 8], F32, tag="cv")
            col_idx = pool_res.tile([128, g * 8], U32, tag="ci")

            def load_2tiles(pool, dram, width, tag):
                t0 = pool.tile([128, width], F32, tag=tag + "0")
                t1 = pool.tile([128, width], F32, tag=tag + "1")
                nc.sync.dma_start(out=t0, in_=dram[0:128, :])
                nc.sync.dma_start(out=t1, in_=dram[128:256, :])
                return t0, t1

            def normalize(t0, t1, width, nrm_tile, scale_engine):
                """Column-L2-normalize the pair of c-tiles in place.
                nrm_tile: [128, width] scratch (partition-broadcast norms).
                scale_engine: nc.vector or nc.gpsimd for the 2 big scaling ops."""
                for ck in range(0, width, CHUNK):
                    w = min(CHUNK, width - ck)
                    ts0 = pool_sq.tile([128, CHUNK], F32, tag="ts0")
                    ts1 = pool_sq.tile([128, CHUNK], F32, tag="ts1")
                    nc.scalar.activation(
                        ts0[:, :w], t0[:, ck : ck + w], mybir.ActivationFunctionType.Square
                    )
                    nc.scalar.activation(
                        ts1[:, :w], t1[:, ck : ck + w], mybir.ActivationFunctionType.Square
                    )
                    ps = pool_psum.tile([128, CHUNK], F32, tag="ps")
                    nc.tensor.matmul(ps[:, :w], ones, ts0[:, :w], start=True, stop=False)
                    nc.tensor.matmul(ps[:, :w], ones, ts1[:, :w], start=False, stop=True)
                    # norm = sqrt(sum sq), broadcast across partitions already
                    nc.scalar.activation(
                        nrm_tile[:, ck : ck + w], ps[:, :w], mybir.ActivationFunctionType.Sqrt
                    )
                nc.vector.reciprocal(nrm_tile, nrm_tile)
                scale_engine.tensor_tensor(t0, t0, nrm_tile, mybir.AluOpType.mult)
                scale_engine.tensor_tensor(t1, t1, nrm_tile, mybir.AluOpType.mult)

            # ---- Phase 0: load + normalize p_full, q_shard ----
            p0, p1 = load_2tiles(pool_stream, p_full, ntot, "str_p")
            qs0, qs1 = load_2tiles(pool_shard, q_shard, ns, "qs")
            ps0, ps1 = load_2tiles(pool_shard, p_shard, ns, "psh")

            nrm_p = pool_block.tile([128, ntot], F32, tag="blk")
            normalize(p0, p1, ntot, nrm_p, nc.vector)
            nrm_qs = pool_small.tile([128, ns], F32, tag="nrm_s")
            normalize(qs0, qs1, ns, nrm_qs, nc.vector)

            # ---- GEMM1: sim rows = q_shard^T @ p_full ----
            def gemm_scan(w0, w1, s0, s1, vals, idxs, dma_sim):
                """w: [128, ns] weight c-tiles; s: [128, ntot] stream c-tiles."""
                for gi in range(g):
                    blk = pool_block.tile([128, ntot], F32, tag="blk")
                    lhs0 = w0[:, gi * 128 : (gi + 1) * 128]
                    lhs1 = w1[:, gi * 128 : (gi + 1) * 128]
                    for ck in range(nchunk_full):
                        w = min(CHUNK, ntot - ck * CHUNK)
                        ps = pool_psum.tile([128, CHUNK], F32, tag="ps")
                        nc.tensor.matmul(
                            ps[:, :w], lhs0, s0[:, ck * CHUNK : ck * CHUNK + w],
                            start=True, stop=False,
                        )
                        nc.tensor.matmul(
                            ps[:, :w], lhs1, s1[:, ck * CHUNK : ck * CHUNK + w],
                            start=False, stop=True,
                        )
                        # evict PSUM -> SBUF (only ACT/DVE can read PSUM)
                        nc.scalar.activation(
                            blk[:, ck * CHUNK : ck * CHUNK + w], ps[:, :w],
                            mybir.ActivationFunctionType.Copy,
                        )
                    if dma_sim is not None:
                        nc.sync.dma_start(
                            out=dma_sim[gi * 128 : (gi + 1) * 128, :], in_=blk
                        )
                    nc.vector.max(out=vals[:, gi * 8 : (gi + 1) * 8], in_=blk)
                    nc.vector.max_index(
                        out=idxs[:, gi * 8 : (gi + 1) * 8],
                        in_max=vals[:, gi * 8 : (gi + 1) * 8],
                        in_values=blk,
                    )

            gemm_scan(qs0, qs1, p0, p1, row_vals, row_idx, sim_out)
            nc.sync.dma_start(out=row_vals_o[:], in_=row_vals)
            nc.sync.dma_start(out=row_idx_o[:], in_=row_idx)

            # ---- Phase 2: load + normalize q_full, p_shard; GEMM2 ----
            q0, q1 = load_2tiles(pool_stream, q_full, ntot, "str_p")
            nrm_q = pool_block.tile([128, ntot], F32, tag="blk")
            normalize(q0, q1, ntot, nrm_q, nc.gpsimd)
            nrm_ps = pool_small.tile([128, ns], F32, tag="nrm_s")
            normalize(ps0, ps1, ns, nrm_ps, nc.vector)

            gemm_scan(ps0, ps1, q0, q1, col_vals, col_idx, None)
            nc.sync.dma_start(out=col_vals_o[:], in_=col_vals)
            nc.sync.dma_start(out=col_idx_o[:], in_=col_idx)

    if do_compile:
        nc.compile()
    return nc


_CACHED = {}


def _get_graph():
    if "nc" not in _CACHED:
        _CACHED["nc"] = build_graph()
    return _CACHED["nc"]


def _unpack_idx(arr, g=G):
    """[128, g*8] per-group slot-0 -> [g*128] in shard row order."""
    return arr.reshape(128, g, 8)[:, :, 0].T.reshape(g * 128)


def kernel(feature_query: np.ndarray, feature_projection: np.ndarray):
    from concourse.bass_utils import run_bass_kernel_spmd

    q = np.ascontiguousarray(np.asarray(feature_query, np.float32).reshape(C, NTOT))
    p = np.ascontiguousarray(np.asarray(feature_projection, np.float32).reshape(C, NTOT))

    in_maps = []
    for k in range(NCORES):
        sl = slice(k * NS, (k + 1) * NS)
        in_maps.append(
            {
                "q_full": q,
                "p_full": p,
                "q_shard": np.ascontiguousarray(q[:, sl]),
                "p_shard": np.ascontiguousarray(p[:, sl]),
            }
        )

    nc = _get_graph()
    res = run_bass_kernel_spmd(nc, in_maps, core_ids=list(range(NCORES))).results

    sim = np.concatenate([res[k]["sim"] for k in range(NCORES)], axis=0)
    row_max = np.concatenate(
        [_unpack_idx(res[k]["row_idx"]) for k in range(NCORES)]
    ).astype(np.int64)
    q_idx = np.concatenate(
        [_unpack_idx(res[k]["col_idx"]) for k in range(NCORES)]
    ).astype(np.int64)
    sim_vals = np.concatenate(
        [res[k]["col_vals"].reshape(128, G, 8)[:, :, 0].T.reshape(NS) for k in range(NCORES)]
    )

    j = np.arange(NTOT)
    mutual = (row_max[q_idx] == j) & (sim_vals > THRESHOLD)
    q_div, q_mod = q_idx // W, q_idx % W
    p_div, p_mod = j // W, j % W
    valid = mutual & (q_div + 1 < H) & (p_div + 1 < H)

    i32 = np.int32
    return (
        sim,
        valid,
        q_idx.astype(i32),
        q_div.astype(i32),
        q_mod.astype(i32),
        p_div.astype(i32),
        p_mod.astype(i32),
    )


# revision 17
# speedup vs baseline: 1.3332x; 1.3332x over previous
"""DenseMatcher kernel for 8 TRN2 NeuronCores.

sim = (q/|q|)^T (p/|p|)  [9216, 9216], row-argmax, col-argmax, mutual-NN.

Sharding: core k owns sim rows [k*1152, (k+1)*1152) (GEMM1: q_shard^T @ p_full)
and sim cols [k*1152, (k+1)*1152) (GEMM2: p_shard^T @ q_full, whose row-argmax
is the col-argmax of sim). All heavy compute (normalize, both GEMMs, argmax
scans) on device; host only slices inputs, concatenates per-core results and
does the O(M) mutual-NN index arithmetic.
"""

import numpy as np
import sys

sys.path.insert(0, "/opt/trn_rl_repo")

import concourse.bacc as bacc
import concourse.bass as bass
import concourse.mybir as mybir
from concourse.tile import TileContext

F32 = mybir.dt.float32
F32R = mybir.dt.float32r
U32 = mybir.dt.uint32

C = 256  # channels (2 partition tiles of 128)
H = 96
W = 96
NTOT = H * W  # 9216
NCORES = 8
NS = NTOT // NCORES  # 1152 rows/cols per core
G = NS // 128  # 9 groups of 128 rows
CHUNK = 512
NCHUNK = NTOT // CHUNK  # 18
THRESHOLD = 0.9


def build_graph(ntot=NTOT, ns=NS, do_compile=True):
    """Build the per-core SPMD graph. Parameterized so a scaled-down version
    can run under the interpreter for shape checking."""
    g = ns // 128
    nchunk_full = (ntot + CHUNK - 1) // CHUNK

    nc = bacc.Bacc("TRN2", target_bir_lowering=False, debug=False)
    q_full = nc.declare_dram_parameter("q_full", [C, ntot], F32, isOutput=False)
    p_full = nc.declare_dram_parameter("p_full", [C, ntot], F32, isOutput=False)
    q_shard = nc.declare_dram_parameter("q_shard", [C, ns], F32, isOutput=False)
    p_shard = nc.declare_dram_parameter("p_shard", [C, ns], F32, isOutput=False)

    sim_out = nc.declare_dram_parameter("sim", [ns, ntot], F32, isOutput=True)
    row_vals_o = nc.declare_dram_parameter("row_vals", [128, g * 8], F32, isOutput=True)
    row_idx_o = nc.declare_dram_parameter("row_idx", [128, g * 8], U32, isOutput=True)
    col_vals_o = nc.declare_dram_parameter("col_vals", [128, g * 8], F32, isOutput=True)
    col_idx_o = nc.declare_dram_parameter("col_idx", [128, g * 8], U32, isOutput=True)

    with TileContext(nc) as tc:
        with (
            tc.tile_pool(name="stream", bufs=1) as pool_stream,
            tc.tile_pool(name="block", bufs=2) as pool_block,
            tc.tile_pool(name="shard", bufs=1) as pool_shard,
            tc.tile_pool(name="small", bufs=2) as pool_small,
            tc.tile_pool(name="sq", bufs=2) as pool_sq,
            tc.tile_pool(name="res", bufs=1) as pool_res,
            tc.tile_pool(name="psum", bufs=8, space="PSUM") as pool_psum,
            tc.tile_pool(name="const", bufs=1) as pool_const,
        ):
            ones = pool_const.tile([128, 128], F32, tag="ones")
            nc.vector.memset(ones, 1.0)

            # result tiles (persist across group loops)
            row_vals = pool_res.tile([128, g * 8], F32, tag="rv")
            row_idx = pool_res.tile([128, g * 8], U32, tag="ri")
            col_vals = pool_res.tile([128, g * 8], F32, tag="cv")
            col_idx = pool_res.tile([128, g * 8], U32, tag="ci")

            def load_2tiles(pool, dram, width, tag):
                # float32r tiles: the gpsimd casting DMA rounds f32 -> f32r on
                # load, so every writer of a matmul input is fp32r-rounded
                # (BIR verifier requirement). Non-matmul readers bitcast to F32.
                t0 = pool.tile([128, width], F32R, tag=tag + "0")
                t1 = pool.tile([128, width], F32R, tag=tag + "1")
                nc.gpsimd.dma_start(out=t0, in_=dram[0:128, :])
                nc.gpsimd.dma_start(out=t1, in_=dram[128:256, :])
                return t0, t1

            def normalize(t0, t1, width, nrm_tile, scale_engine):
                """Column-L2-normalize the pair of c-tiles in place.
                nrm_tile: [128, width] scratch (partition-broadcast norms).
                scale_engine: nc.vector or nc.gpsimd for the 2 big scaling ops."""
                for ck in range(0, width, CHUNK):
                    w = min(CHUNK, width - ck)
                    ts0 = pool_sq.tile([128, CHUNK], F32, tag="ts0")
                    ts1 = pool_sq.tile([128, CHUNK], F32, tag="ts1")
                    nc.scalar.activation(
                        ts0[:, :w],
                        t0[:, ck : ck + w].bitcast(F32),
                        mybir.ActivationFunctionType.Square,
                    )
                    nc.scalar.activation(
                        ts1[:, :w],
                        t1[:, ck : ck + w].bitcast(F32),
                        mybir.ActivationFunctionType.Square,
                    )
                    ps = pool_psum.tile([128, CHUNK], F32, tag="ps")
                    nc.tensor.matmul(ps[:, :w], ones, ts0[:, :w], start=True, stop=False)
                    nc.tensor.matmul(ps[:, :w], ones, ts1[:, :w], start=False, stop=True)
                    # norm = sqrt(sum sq), broadcast across partitions already
                    nc.scalar.activation(
                        nrm_tile[:, ck : ck + w], ps[:, :w], mybir.ActivationFunctionType.Sqrt
                    )
                nc.vector.reciprocal(nrm_tile, nrm_tile)
                # in-place scale; output dtype f32r -> ALU rounds on write
                scale_engine.tensor_tensor(
                    t0, t0.bitcast(F32), nrm_tile, mybir.AluOpType.mult
                )
                scale_engine.tensor_tensor(
                    t1, t1.bitcast(F32), nrm_tile, mybir.AluOpType.mult
                )

            # ---- Phase 0: load + normalize p_full, q_shard ----
            p0, p1 = load_2tiles(pool_stream, p_full, ntot, "str_p")
            qs0, qs1 = load_2tiles(pool_shard, q_shard, ns, "qs")
            ps0, ps1 = load_2tiles(pool_shard, p_shard, ns, "psh")

            nrm_p = pool_block.tile([128, ntot], F32, tag="blk")
            normalize(p0, p1, ntot, nrm_p, nc.vector)
            nrm_qs = pool_small.tile([128, ns], F32, tag="nrm_s")
            normalize(qs0, qs1, ns, nrm_qs, nc.vector)

            # ---- GEMM1: sim rows = q_shard^T @ p_full ----
            def gemm_scan(w0, w1, s0, s1, vals, idxs, dma_sim):
                """w: [128, ns] weight c-tiles; s: [128, ntot] stream c-tiles."""
                for gi in range(g):
                    blk = pool_block.tile([128, ntot], F32, tag="blk")
                    lhs0 = w0[:, gi * 128 : (gi + 1) * 128]
                    lhs1 = w1[:, gi * 128 : (gi + 1) * 128]
                    for ck in range(nchunk_full):
                        w = min(CHUNK, ntot - ck * CHUNK)
                        ps = pool_psum.tile([128, CHUNK], F32, tag="ps")
                        nc.tensor.matmul(
                            ps[:, :w], lhs0, s0[:, ck * CHUNK : ck * CHUNK + w],
                            start=True, stop=False,
                        )
                        nc.tensor.matmul(
                            ps[:, :w], lhs1, s1[:, ck * CHUNK : ck * CHUNK + w],
                            start=False, stop=True,
                        )
                        # evict PSUM -> SBUF (only ACT/DVE can read PSUM)
                        nc.scalar.activation(
                            blk[:, ck * CHUNK : ck * CHUNK + w], ps[:, :w],
                            mybir.ActivationFunctionType.Copy,
                        )
                    if dma_sim is not None:
                        nc.sync.dma_start(
                            out=dma_sim[gi * 128 : (gi + 1) * 128, :], in_=blk
                        )
                    nc.vector.max(out=vals[:, gi * 8 : (gi + 1) * 8], in_=blk)
                    nc.vector.max_index(
                        out=idxs[:, gi * 8 : (gi + 1) * 8],
                        in_max=vals[:, gi * 8 : (gi + 1) * 8],
                        in_values=blk,
                    )

            gemm_scan(qs0, qs1, p0, p1, row_vals, row_idx, sim_out)
            nc.sync.dma_start(out=row_vals_o[:], in_=row_vals)
            nc.sync.dma_start(out=row_idx_o[:], in_=row_idx)

            # ---- Phase 2: load + normalize q_full, p_shard; GEMM2 ----
            q0, q1 = load_2tiles(pool_stream, q_full, ntot, "str_p")
            nrm_q = pool_block.tile([128, ntot], F32, tag="blk")
            normalize(q0, q1, ntot, nrm_q, nc.gpsimd)
            nrm_ps = pool_small.tile([128, ns], F32, tag="nrm_s")
            normalize(ps0, ps1, ns, nrm_ps, nc.vector)

            gemm_scan(ps0, ps1, q0, q1, col_vals, col_idx, None)
            nc.sync.dma_start(out=col_vals_o[:], in_=col_vals)
            nc.sync.dma_start(out=col_idx_o[:], in_=col_idx)

    if do_compile:
        nc.compile()
    return nc


_CACHED = {}


def _get_graph():
    if "nc" not in _CACHED:
        _CACHED["nc"] = build_graph()
    return _CACHED["nc"]


def _unpack8(arr, g=G):
    """[128, g*8] -> [g*128, 8] candidate slots in shard row order."""
    return arr.reshape(128, g, 8).transpose(1, 0, 2).reshape(g * 128, 8)


def _refine_argmax(cand_idx, qn, pn, transpose):
    """Exact f64 rescoring of the top-8 fp32r candidates per row.

    cand_idx: [R, 8] candidate indices. For rows (transpose=False) row r of sim
    scores candidates as qn[:, r] . pn[:, cand]; for cols the roles swap.
    Returns (argmax_index [R], max_value [R])."""
    a, b = (qn, pn) if not transpose else (pn, qn)
    # scores[r, k] = a[:, r] . b[:, cand_idx[r, k]]
    scores = np.einsum("cr,crk->rk", a, b[:, cand_idx], optimize=True)
    # pick max score; break exact ties by smallest index (jnp argmax semantics)
    best = scores.max(axis=1, keepdims=True)
    masked = np.where(scores == best, cand_idx, np.iinfo(np.int64).max)
    idx = masked.min(axis=1)
    return idx, best[:, 0]


def kernel(feature_query: np.ndarray, feature_projection: np.ndarray):
    from concourse.bass_utils import run_bass_kernel_spmd

    q = np.ascontiguousarray(np.asarray(feature_query, np.float32).reshape(C, NTOT))
    p = np.ascontiguousarray(np.asarray(feature_projection, np.float32).reshape(C, NTOT))

    in_maps = []
    for k in range(NCORES):
        sl = slice(k * NS, (k + 1) * NS)
        in_maps.append(
            {
                "q_full": q,
                "p_full": p,
                "q_shard": np.ascontiguousarray(q[:, sl]),
                "p_shard": np.ascontiguousarray(p[:, sl]),
            }
        )

    nc = _get_graph()
    res = run_bass_kernel_spmd(nc, in_maps, core_ids=list(range(NCORES))).results

    sim = np.concatenate([res[k]["sim"] for k in range(NCORES)], axis=0)
    row_cand = np.concatenate(
        [_unpack8(res[k]["row_idx"]) for k in range(NCORES)]
    ).astype(np.int64)
    col_cand = np.concatenate(
        [_unpack8(res[k]["col_idx"]) for k in range(NCORES)]
    ).astype(np.int64)

    # exact f64 rescoring of the device's top-8 candidates (fp32r GEMM noise
    # ~1e-4 << top-2 gap median 1.3e-2, so the true argmax is in the top-8)
    q64 = q.astype(np.float64)
    p64 = p.astype(np.float64)
    qn = q64 / np.linalg.norm(q64, axis=0, keepdims=True)
    pn = p64 / np.linalg.norm(p64, axis=0, keepdims=True)
    row_max, _ = _refine_argmax(row_cand, qn, pn, transpose=False)
    q_idx, sim_vals = _refine_argmax(col_cand, qn, pn, transpose=True)

    j = np.arange(NTOT)
    mutual = (row_max[q_idx] == j) & (sim_vals > THRESHOLD)
    q_div, q_mod = q_idx // W, q_idx % W
    p_div, p_mod = j // W, j % W
    valid = mutual & (q_div + 1 < H) & (p_div + 1 < H)

    i32 = np.int32
    return (
        sim,
        valid,
        q_idx.astype(i32),
        q_div.astype(i32),
        q_mod.astype(i32),
        p_div.astype(i32),
        p_mod.astype(i32),
    )


# revision 23
# speedup vs baseline: 1.4279x; 1.0711x over previous
"""DenseMatcher kernel for 8 TRN2 NeuronCores.

sim = (q/|q|)^T (p/|p|)  [9216, 9216], row-argmax, col-argmax, mutual-NN.

Sharding: core k owns sim rows [k*1152, (k+1)*1152) (GEMM1: q_shard^T @ p_full)
and sim cols [k*1152, (k+1)*1152) (GEMM2: p_shard^T @ q_full, whose row-argmax
is the col-argmax of sim). All heavy compute (normalize, both GEMMs, argmax
scans) on device; host only slices inputs, concatenates per-core results and
does the O(M) mutual-NN index arithmetic.
"""

import numpy as np
import sys

sys.path.insert(0, "/opt/trn_rl_repo")

import concourse.bacc as bacc
import concourse.bass as bass
import concourse.mybir as mybir
from concourse.tile import TileContext

F32 = mybir.dt.float32
F32R = mybir.dt.float32r
U32 = mybir.dt.uint32

C = 256  # channels (2 partition tiles of 128)
H = 96
W = 96
NTOT = H * W  # 9216
NCORES = 8
NS = NTOT // NCORES  # 1152 rows/cols per core
G = NS // 128  # 9 groups of 128 rows
CHUNK = 512
NCHUNK = NTOT // CHUNK  # 18
THRESHOLD = 0.9


def build_graph(ntot=NTOT, ns=NS, do_compile=True):
    """Build the per-core SPMD graph. Parameterized so a scaled-down version
    can run under the interpreter for shape checking."""
    g = ns // 128
    nchunk_full = (ntot + CHUNK - 1) // CHUNK

    nc = bacc.Bacc("TRN2", target_bir_lowering=False, debug=False)
    q_full = nc.declare_dram_parameter("q_full", [C, ntot], F32, isOutput=False)
    p_full = nc.declare_dram_parameter("p_full", [C, ntot], F32, isOutput=False)
    q_shard = nc.declare_dram_parameter("q_shard", [C, ns], F32, isOutput=False)
    p_shard = nc.declare_dram_parameter("p_shard", [C, ns], F32, isOutput=False)

    sim_out = nc.declare_dram_parameter("sim", [ns, ntot], F32, isOutput=True)
    row_vals_o = nc.declare_dram_parameter("row_vals", [128, g * 8], F32, isOutput=True)
    row_idx_o = nc.declare_dram_parameter("row_idx", [128, g * 8], U32, isOutput=True)
    col_vals_o = nc.declare_dram_parameter("col_vals", [128, g * 8], F32, isOutput=True)
    col_idx_o = nc.declare_dram_parameter("col_idx", [128, g * 8], U32, isOutput=True)

    with TileContext(nc) as tc:
        with (
            tc.tile_pool(name="stream", bufs=1) as pool_stream,
            tc.tile_pool(name="block", bufs=2) as pool_block,
            tc.tile_pool(name="shard", bufs=1) as pool_shard,
            tc.tile_pool(name="small", bufs=2) as pool_small,
            tc.tile_pool(name="sq", bufs=2) as pool_sq,
            tc.tile_pool(name="res", bufs=1) as pool_res,
            tc.tile_pool(name="psum", bufs=4, space="PSUM") as pool_psum,
            tc.tile_pool(name="const", bufs=1) as pool_const,
        ):
            ones = pool_const.tile([128, 128], F32, tag="ones")
            nc.vector.memset(ones, 1.0)

            # result tiles (persist across group loops)
            row_vals = pool_res.tile([128, g * 8], F32, tag="rv")
            row_idx = pool_res.tile([128, g * 8], U32, tag="ri")
            col_vals = pool_res.tile([128, g * 8], F32, tag="cv")
            col_idx = pool_res.tile([128, g * 8], U32, tag="ci")

            def load_2tiles(pool, dram, width, tag):
                # float32r tiles: the gpsimd casting DMA rounds f32 -> f32r on
                # load, so every writer of a matmul input is fp32r-rounded
                # (BIR verifier requirement). Non-matmul readers bitcast to F32.
                t0 = pool.tile([128, width], F32R, tag=tag + "0")
                t1 = pool.tile([128, width], F32R, tag=tag + "1")
                nc.gpsimd.dma_start(out=t0, in_=dram[0:128, :])
                nc.gpsimd.dma_start(out=t1, in_=dram[128:256, :])
                return t0, t1

            def normalize(t0, t1, width, nrm_tile, scratch_tile, scale_engine):
                """Column-L2-normalize the pair of c-tiles in place.
                nrm_tile: [128, width] scratch (partition-broadcast norms).
                scale_engine: nc.vector or nc.gpsimd for the 2 big scaling ops."""
                for ck in range(0, width, CHUNK):
                    w = min(CHUNK, width - ck)
                    ts0 = pool_sq.tile([128, CHUNK], F32, tag="ts0")
                    ts1 = pool_sq.tile([128, CHUNK], F32, tag="ts1")
                    nc.scalar.activation(
                        ts0[:, :w],
                        t0[:, ck : ck + w].bitcast(F32),
                        mybir.ActivationFunctionType.Square,
                    )
                    nc.scalar.activation(
                        ts1[:, :w],
                        t1[:, ck : ck + w].bitcast(F32),
                        mybir.ActivationFunctionType.Square,
                    )
                    ps = pool_psum.tile([128, CHUNK], F32, tag="ps")
                    nc.tensor.matmul(ps[:, :w], ones, ts0[:, :w], start=True, stop=False)
                    nc.tensor.matmul(ps[:, :w], ones, ts1[:, :w], start=False, stop=True)
                    # norm = sqrt(sum sq), broadcast across partitions already
                    nc.scalar.activation(
                        nrm_tile[:, ck : ck + w], ps[:, :w], mybir.ActivationFunctionType.Sqrt
                    )
                nc.vector.reciprocal_approx_accurate(nrm_tile, nrm_tile, scratch_tile)
                # in-place scale; output dtype f32r -> ALU rounds on write
                scale_engine.tensor_tensor(
                    t0, t0.bitcast(F32), nrm_tile, mybir.AluOpType.mult
                )
                scale_engine.tensor_tensor(
                    t1, t1.bitcast(F32), nrm_tile, mybir.AluOpType.mult
                )

            # ---- Phase 0: load + normalize p_full, q_shard ----
            p0, p1 = load_2tiles(pool_stream, p_full, ntot, "str_p")
            qs0, qs1 = load_2tiles(pool_shard, q_shard, ns, "qs")
            ps0, ps1 = load_2tiles(pool_shard, p_shard, ns, "psh")

            nrm_p = pool_block.tile([128, ntot], F32, tag="blk")
            scr_p = pool_block.tile([128, ntot], F32, tag="blk")
            normalize(p0, p1, ntot, nrm_p, scr_p, nc.gpsimd)
            nrm_qs = pool_small.tile([128, ns], F32, tag="nrm_s")
            scr_qs = pool_small.tile([128, ns], F32, tag="nrm_s")
            normalize(qs0, qs1, ns, nrm_qs, scr_qs, nc.gpsimd)

            # ---- GEMM1: sim rows = q_shard^T @ p_full ----
            def gemm_scan(w0, w1, s0, s1, vals, idxs, dma_sim):
                """w: [128, ns] weight c-tiles; s: [128, ntot] stream c-tiles."""
                assert ntot % (2 * CHUNK) == 0
                for gi in range(g):
                    blk = pool_block.tile([128, ntot], F32, tag="blk")
                    lhs0 = w0[:, gi * 128 : (gi + 1) * 128]
                    lhs1 = w1[:, gi * 128 : (gi + 1) * 128]
                    # pairs of chunks share a 2-bank PSUM tile (one ACT evict
                    # per pair); weights loaded once per c-tile per pair-block
                    for cb in range(0, nchunk_full, 2):
                        ps = pool_psum.tile([128, 2 * CHUNK], F32, tag="ps")
                        for lhs, s, st in ((lhs0, s0, True), (lhs1, s1, False)):
                            for i in (0, 1):
                                ck = cb + i
                                nc.tensor.matmul(
                                    ps[:, i * CHUNK : (i + 1) * CHUNK],
                                    lhs,
                                    s[:, ck * CHUNK : (ck + 1) * CHUNK],
                                    start=st, stop=not st,
                                )
                        nc.scalar.activation(
                            blk[:, cb * CHUNK : (cb + 2) * CHUNK], ps,
                            mybir.ActivationFunctionType.Copy,
                        )
                    if dma_sim is not None:
                        nc.sync.dma_start(
                            out=dma_sim[gi * 128 : (gi + 1) * 128, :], in_=blk
                        )
                    nc.vector.max(out=vals[:, gi * 8 : (gi + 1) * 8], in_=blk)
                    nc.vector.max_index(
                        out=idxs[:, gi * 8 : (gi + 1) * 8],
                        in_max=vals[:, gi * 8 : (gi + 1) * 8],
                        in_values=blk,
                    )

            gemm_scan(qs0, qs1, p0, p1, row_vals, row_idx, sim_out)
            nc.sync.dma_start(out=row_vals_o[:], in_=row_vals)
            nc.sync.dma_start(out=row_idx_o[:], in_=row_idx)

            # ---- Phase 2: load + normalize q_full, p_shard; GEMM2 ----
            q0, q1 = load_2tiles(pool_stream, q_full, ntot, "str_p")
            nrm_q = pool_block.tile([128, ntot], F32, tag="blk")
            scr_q = pool_block.tile([128, ntot], F32, tag="blk")
            normalize(q0, q1, ntot, nrm_q, scr_q, nc.gpsimd)
            nrm_ps = pool_small.tile([128, ns], F32, tag="nrm_s")
            scr_ps = pool_small.tile([128, ns], F32, tag="nrm_s")
            normalize(ps0, ps1, ns, nrm_ps, scr_ps, nc.gpsimd)

            gemm_scan(ps0, ps1, q0, q1, col_vals, col_idx, None)
            nc.sync.dma_start(out=col_vals_o[:], in_=col_vals)
            nc.sync.dma_start(out=col_idx_o[:], in_=col_idx)

    if do_compile:
        nc.compile()
    return nc


_CACHED = {}


def _get_graph():
    if "nc" not in _CACHED:
        _CACHED["nc"] = build_graph()
    return _CACHED["nc"]


def _unpack8(arr, g=G):
    """[128, g*8] -> [g*128, 8] candidate slots in shard row order."""
    return arr.reshape(128, g, 8).transpose(1, 0, 2).reshape(g * 128, 8)


def _refine_argmax(cand_idx, qn, pn, transpose):
    """Exact f64 rescoring of the top-8 fp32r candidates per row.

    cand_idx: [R, 8] candidate indices. For rows (transpose=False) row r of sim
    scores candidates as qn[:, r] . pn[:, cand]; for cols the roles swap.
    Returns (argmax_index [R], max_value [R])."""
    a, b = (qn, pn) if not transpose else (pn, qn)
    # scores[r, k] = a[:, r] . b[:, cand_idx[r, k]]
    scores = np.einsum("cr,crk->rk", a, b[:, cand_idx], optimize=True)
    # pick max score; break exact ties by smallest index (jnp argmax semantics)
    best = scores.max(axis=1, keepdims=True)
    masked = np.where(scores == best, cand_idx, np.iinfo(np.int64).max)
    idx = masked.min(axis=1)
    return idx, best[:, 0]


def kernel(feature_query: np.ndarray, feature_projection: np.ndarray):
    from concourse.bass_utils import run_bass_kernel_spmd

    q = np.ascontiguousarray(np.asarray(feature_query, np.float32).reshape(C, NTOT))
    p = np.ascontiguousarray(np.asarray(feature_projection, np.float32).reshape(C, NTOT))

    in_maps = []
    for k in range(NCORES):
        sl = slice(k * NS, (k + 1) * NS)
        in_maps.append(
            {
                "q_full": q,
                "p_full": p,
                "q_shard": np.ascontiguousarray(q[:, sl]),
                "p_shard": np.ascontiguousarray(p[:, sl]),
            }
        )

    nc = _get_graph()
    res = run_bass_kernel_spmd(nc, in_maps, core_ids=list(range(NCORES))).results

    sim = np.concatenate([res[k]["sim"] for k in range(NCORES)], axis=0)
    row_cand = np.concatenate(
        [_unpack8(res[k]["row_idx"]) for k in range(NCORES)]
    ).astype(np.int64)
    col_cand = np.concatenate(
        [_unpack8(res[k]["col_idx"]) for k in range(NCORES)]
    ).astype(np.int64)

    # exact f64 rescoring of the device's top-8 candidates (fp32r GEMM noise
    # ~1e-4 << top-2 gap median 1.3e-2, so the true argmax is in the top-8)
    q64 = q.astype(np.float64)
    p64 = p.astype(np.float64)
    qn = q64 / np.linalg.norm(q64, axis=0, keepdims=True)
    pn = p64 / np.linalg.norm(p64, axis=0, keepdims=True)
    row_max, _ = _refine_argmax(row_cand, qn, pn, transpose=False)
    q_idx, sim_vals = _refine_argmax(col_cand, qn, pn, transpose=True)

    j = np.arange(NTOT)
    mutual = (row_max[q_idx] == j) & (sim_vals > THRESHOLD)
    q_div, q_mod = q_idx // W, q_idx % W
    p_div, p_mod = j // W, j % W
    valid = mutual & (q_div + 1 < H) & (p_div + 1 < H)

    i32 = np.int32
    return (
        sim,
        valid,
        q_idx.astype(i32),
        q_div.astype(i32),
        q_mod.astype(i32),
        p_div.astype(i32),
        p_mod.astype(i32),
    )
